# revision 6
# baseline (speedup 1.0000x reference)
"""VQ codebook nearest-neighbor kernel for Trainium2 (8 NeuronCores).

Problem: embeddings (16, 4096, 64) f32, codebook (1024, 64) f32.
Output: argmin_j ||e - c_j||^2 -> (16, 4096) int32.

Math: argmin_j (||c_j||^2 - 2 e.c_j) == argmax_j (2 e.c_j - ||c_j||^2).
We fold the ||c||^2 term into the matmul by augmenting the contraction
dim: lhsT row 64 = ones, rhs row 64 = -||c||^2, rhs rows 0-63 = 2*c^T.

Sharding: data-parallel over flattened N = B*S, 8192 rows per core;
codebook replicated.

Per-core kernel (rows on partitions, codes on free dim):
  - 64 row-tiles of 128 rows; per tile two f32r matmuls (K=65, N=512)
    into one (128, 1024) PSUM tile,
  - ScalarE evacuates PSUM -> SBUF,
  - VectorE max (top-8) + max_index -> per-row argmax index (uint32),
  - one strided DMA writes all indices back.
"""

import os
import sys

for _p in ("/opt/trn_rl_repo", "/root/.axon_site/_ro/trn_rl_repo"):
    if os.path.isdir(_p) and _p not in sys.path:
        sys.path.append(_p)

import numpy as np

import concourse.bacc as bacc
import concourse.bass as bass
import concourse.mybir as mybir
from concourse.bass_utils import run_bass_kernel_spmd
from concourse.tile import TileContext

B, S, D = 16, 4096, 64
A = 1024                     # num codes
N_CORES = 8
N_TOTAL = B * S              # 65536
N_PER_CORE = N_TOTAL // N_CORES   # 8192
K = D + 1                    # contraction dim incl. ones row
ROW_TILE = 128
F32 = mybir.dt.float32
U32 = mybir.dt.uint32
BF16 = mybir.dt.bfloat16


def build_nc(n_rows: int = N_PER_CORE, dma_chunks: int = 8) -> bass.Bass:
    """Build the per-core Bass module. All 8 cores run this same program
    on their own shard.

    Scores are computed exactly (fp32-grade) via a bf16 hi/lo split:
      e.c ~= e_hi.c_hi + e_hi.c_lo + e_lo.c_hi  (e_lo.c_lo ~ 2^-18 dropped)
    accumulated in fp32 PSUM across three bf16 matmuls. The ||c||^2 term
    rides the augmented row 64: ones in both e_hi/e_lo; rhs row 64 holds
    three successive bf16 residuals of -||c||^2.
    """
    n_tiles = n_rows // ROW_TILE
    nc = bacc.Bacc()
    et_hi = nc.declare_dram_parameter("et_hi", [K, n_rows], BF16,
                                      isOutput=False)
    et_lo = nc.declare_dram_parameter("et_lo", [K, n_rows], BF16,
                                      isOutput=False)
    cbt = nc.declare_dram_parameter("cbt", [3, K, A], BF16, isOutput=False)
    idx = nc.declare_dram_parameter("idx", [n_rows], U32, isOutput=True)

    with TileContext(nc) as tc:
        with (
            tc.tile_pool(name="const", bufs=1) as const_pool,
            tc.tile_pool(name="etp", bufs=2 * dma_chunks) as et_pool,
            tc.tile_pool(name="ps", bufs=3, space="PSUM") as psum_pool,
            tc.tile_pool(name="sc", bufs=3) as sc_pool,
            tc.tile_pool(name="m8", bufs=3) as m8_pool,
        ):
            cb_tile = const_pool.tile([K, 3 * A], BF16)
            cb_view = cb_tile.rearrange("k (s a) -> k s a", s=3)
            nc.sync.dma_start(out=cb_view, in_=cbt.rearrange("s k a -> k s a"))
            # all per-row results staged here, written out once at the end
            stage = const_pool.tile([ROW_TILE, n_tiles * 8], U32)

            cols_per_chunk = n_rows // dma_chunks
            tiles_per_chunk = cols_per_chunk // ROW_TILE
            e_tiles = []
            for ci in range(dma_chunks):
                sl = slice(ci * cols_per_chunk, (ci + 1) * cols_per_chunk)
                thi = et_pool.tile([K, cols_per_chunk], BF16, tag="ehi")
                nc.sync.dma_start(out=thi, in_=et_hi[:, sl])
                tlo = et_pool.tile([K, cols_per_chunk], BF16, tag="elo")
                nc.sync.dma_start(out=tlo, in_=et_lo[:, sl])
                e_tiles.append((thi, tlo))

            for ti in range(n_tiles):
                ci, local = divmod(ti, tiles_per_chunk)
                csl = slice(local * ROW_TILE, (local + 1) * ROW_TILE)
                lhs_hi = e_tiles[ci][0][:, csl]
                lhs_lo = e_tiles[ci][1][:, csl]
                ps = psum_pool.tile([ROW_TILE, A], F32)
                for h in range(2):
                    hs = slice(h * 512, (h + 1) * 512)
                    # pass 1: e_hi . c_hi ; pass 2: e_hi . c_lo ;
                    # pass 3: e_lo . c_hi3 (row64 = 3rd cbsq residual)
                    for p, lhs in ((0, lhs_hi), (1, lhs_hi), (2, lhs_lo)):
                        nc.tensor.matmul(
                            ps[:, hs],
                            lhs,
                            cb_view[:, p, hs],
                            start=(p == 0),
                            stop=(p == 2),
                        )
                sc = sc_pool.tile([ROW_TILE, A], F32)
                nc.scalar.copy(out=sc[:, :], in_=ps[:, :])
                m8 = m8_pool.tile([ROW_TILE, 8], F32)
                nc.vector.max(out=m8[:, :], in_=sc[:, :])
                nc.vector.max_index(
                    out=stage[:, ti * 8:(ti + 1) * 8],
                    in_max=m8[:, :],
                    in_values=sc[:, :],
                )

            # idx[ti*128 + p] = stage[p, ti*8]
            idx_view = idx.rearrange("(t p) -> p t", p=ROW_TILE)
            src = stage.rearrange("p (t e) -> p t e", e=8)[:, :, 0]
            nc.sync.dma_start(out=idx_view, in_=src)
    nc.compile()
    return nc


def _bf16_split(x64: np.ndarray, n: int):
    """Successive bf16 residuals: sum(parts) ~= x to ~2^-(9n) relative."""
    import ml_dtypes
    parts = []
    resid = x64.astype(np.float64)
    for _ in range(n):
        p = resid.astype(np.float32).astype(ml_dtypes.bfloat16)
        parts.append(p)
        resid = resid - p.astype(np.float64)
    return parts


def make_in_maps(embeddings: np.ndarray, codebook: np.ndarray,
                 n_rows: int = N_PER_CORE, n_cores: int = N_CORES):
    """Host-side sharding/layout prep."""
    import ml_dtypes
    flat = np.asarray(embeddings, dtype=np.float32).reshape(-1, D)
    cb = np.asarray(codebook, dtype=np.float32)

    # rhs rows 0..63 = 2*c^T split hi/lo; row 64 = -||c||^2 split 3-ways
    two_ct = 2.0 * cb.T.astype(np.float64)                    # (D, A)
    ct_hi, ct_lo, ct_r3 = _bf16_split(two_ct, 3)
    cbsq = (cb.astype(np.float64) ** 2).sum(axis=1)           # (A,)
    q_hi, q_lo, q_r3 = _bf16_split(-cbsq, 3)
    cbt = np.zeros((3, K, A), dtype=ml_dtypes.bfloat16)
    cbt[0, :D] = ct_hi
    cbt[1, :D] = ct_lo
    cbt[2, :D] = ct_hi          # pass 3 reuses c_hi for e_lo . c_hi
    cbt[0, D] = q_hi
    cbt[1, D] = q_lo
    cbt[2, D] = q_r3

    e64 = flat.T.astype(np.float64)                           # (D, N)
    e_hi, e_lo = _bf16_split(e64, 2)
    et_hi = np.ones((K, flat.shape[0]), dtype=ml_dtypes.bfloat16)
    et_lo = np.ones((K, flat.shape[0]), dtype=ml_dtypes.bfloat16)
    et_hi[:D] = e_hi
    et_lo[:D] = e_lo            # row 64 stays 1.0 (pairs with q_r3)

    in_maps = []
    for c in range(n_cores):
        sl = slice(c * n_rows, (c + 1) * n_rows)
        in_maps.append({
            "et_hi": np.ascontiguousarray(et_hi[:, sl]),
            "et_lo": np.ascontiguousarray(et_lo[:, sl]),
            "cbt": cbt,
        })
    return in_maps


_NC_CACHE: dict = {}


def _get_nc():
    key = N_PER_CORE
    if key not in _NC_CACHE:
        _NC_CACHE[key] = build_nc()
    return _NC_CACHE[key]


def kernel(embeddings: np.ndarray, codebook: np.ndarray, *,
           trace: bool = False, **run_kwargs) -> np.ndarray:
    nc = _get_nc()
    in_maps = make_in_maps(embeddings, codebook)
    res = run_bass_kernel_spmd(nc, in_maps, core_ids=list(range(N_CORES)),
                               trace=trace, **run_kwargs)
    out = np.concatenate(
        [res.results[c]["idx"].reshape(-1) for c in range(N_CORES)])
    out = out.astype(np.int32).reshape(B, S)
    if trace:
        kernel.last_results = res
    return out
